# revision 37
# baseline (speedup 1.0000x reference)
"""Trainium2 Bass kernel for a dense transformer block (B=64, T=256, C=384, H=6).

Sharding: data-parallel over batch across 8 NeuronCores (8 sequences per
core), no collectives. Each core runs the full block on its shard:
  LN1 -> per-head QKV -> causal attention -> proj (+residual)
  -> LN2 -> FFN relu (+residual)

Layout strategy per core (NTOK = 8*256 = 2048 tokens, 16 row tiles of 128):
  - LN stats via bn_stats/bn_aggr with tokens on partitions.
  - h is PE-transposed to hT [C, NTOK] so QKV matmuls contract over C.
  - q, k are produced transposed ([C_out, NTOK]) with W as the stationary
    operand; v is produced in natural [NTOK, C] layout (it is the stationary
    operand of the attention-value matmul).
  - Scores are computed TRANSPOSED: S_T[s,t] (k stationary, q moving), so
    exp(S_T) is already in the [contraction, free] orientation the value
    matmul needs as its moving operand — no PE transposes / PSUM round-trip
    of the softmax weights at all. The softmax denominator Z[t] is obtained
    in the same PSUM tile via an all-ones [128,64] stationary matmul over
    exp(S_T) (rows 64:128 = Z replicated), reciprocal'd on the DVE, and the
    normalize is fused into the mandatory PSUM->SBUF evacuation.
  - LN gamma/beta are folded into the following weight matrices on the host,
    so on-device LN is a pure standardize.
  - All matmul operands use float32r (tf32-like: 1 cycle/row at N>=256,
    ~1e-4 relative error), fp32 PSUM accumulation, fp32 residual path.
"""
import os
import numpy as np
from contextlib import ExitStack

from concourse import bacc, bass, mybir, tile
from concourse.bass_utils import run_bass_kernel_spmd
from concourse.masks import make_identity

F32 = mybir.dt.float32
I32 = mybir.dt.int32
F32R = getattr(mybir.dt, os.environ.get("MM_DT", "float32r"))
AX = mybir.AxisListType
ALU = mybir.AluOpType
ACT = mybir.ActivationFunctionType

N_CORES = 8
B, T, C, H, D = 64, 256, 384, 6, 64
B_CORE = B // N_CORES          # 8 sequences per core
NTOK = B_CORE * T              # 2048
NT = NTOK // 128               # 16 token tiles
NK = C // 128                  # 3 contraction tiles
NM = C // 128                  # 3 output-column tiles
NCH = NTOK // 512              # 4 column chunks of 512 for [C, NTOK] tensors
EPS = 1e-5
SCALE = 1.0 / float(np.sqrt(np.float32(C)))
NEG = -1e10


def _row_bcast(handle, n):
    """AP that broadcasts a flat [n] DRAM tensor across 128 partitions."""
    ap = handle.ap()
    return bass.AP(tensor=ap.tensor, offset=ap.offset, ap=[[0, 128], [1, n]])


def build(loop_n=None):
    nc = bacc.Bacc("TRN2", target_bir_lowering=False, debug=False,
                   num_devices=N_CORES)

    xin = nc.declare_dram_parameter("x", [B_CORE, T, C], F32, isOutput=False)
    # Wq/Wk/Wv are host-side pre-arranged to [C, H*D] (c-major) so each
    # loads in one 3-dim-AP DMA like the square weights.
    wq = nc.declare_dram_parameter("Wq", [C, C], F32, isOutput=False)
    wk = nc.declare_dram_parameter("Wk", [C, C], F32, isOutput=False)
    wv = nc.declare_dram_parameter("Wv", [C, C], F32, isOutput=False)
    bq = nc.declare_dram_parameter("bq", [H, D], F32, isOutput=False)
    bk = nc.declare_dram_parameter("bk", [H, D], F32, isOutput=False)
    bv = nc.declare_dram_parameter("bv", [H, D], F32, isOutput=False)
    wp = nc.declare_dram_parameter("Wp", [C, C], F32, isOutput=False)
    bp = nc.declare_dram_parameter("bp", [C], F32, isOutput=False)
    w1 = nc.declare_dram_parameter("W1", [C, C], F32, isOutput=False)
    b1 = nc.declare_dram_parameter("b1", [C], F32, isOutput=False)
    w2 = nc.declare_dram_parameter("W2", [C, C], F32, isOutput=False)
    b2 = nc.declare_dram_parameter("b2", [C], F32, isOutput=False)
    yout = nc.declare_dram_parameter("out", [B_CORE, T, C], F32, isOutput=True)

    xf = xin.ap().rearrange("b t c -> (b t) c")
    yf = yout.ap().rearrange("b t c -> (b t) c")

    with tile.TileContext(nc) as tc, ExitStack() as ctx:
        consts = ctx.enter_context(tc.tile_pool(name="consts", bufs=1))
        work = ctx.enter_context(tc.tile_pool(name="work", bufs=1))
        ps = ctx.enter_context(tc.tile_pool(name="ps", bufs=1, space="PSUM"))

        def emit_body():
            # ---- constants -------------------------------------------------
            ident32 = consts.tile([128, 128], F32, tag="ident32")
            make_identity(nc, ident32)
            ident = consts.tile([128, 128], F32R, tag="ident")
            nc.vector.tensor_copy(ident, ident32)

            # Combined additive causal mask [128, 384] in f32r for the
            # TRANSPOSED scores S_T[s,t], applied to the score PSUM via a
            # K=128 identity matmul accumulation. Layout: cols 0:256 =
            # s-tile0 rows (s 0..127 vs t 0..255; keep t >= s), cols
            # 256:384 = s-tile1 rows (s 128..255 vs t 128..255; keep
            # (t-128) >= (s-128)).
            maskst = work.tile([128, 384], F32, tag="maskst", bufs=1)
            nc.gpsimd.memset(maskst, 0.0)
            nc.gpsimd.affine_select(
                out=maskst[:, 0:256], in_=maskst[:, 0:256],
                compare_op=ALU.is_ge, fill=NEG,
                base=0, pattern=[[1, 256]], channel_multiplier=-1)
            nc.gpsimd.affine_select(
                out=maskst[:, 256:384], in_=maskst[:, 256:384],
                compare_op=ALU.is_ge, fill=NEG,
                base=0, pattern=[[1, 128]], channel_multiplier=-1)
            maskF = consts.tile([128, 384], F32R, tag="maskF")
            nc.vector.tensor_copy(maskF, maskst)



            def ln_stats_chunk(src_tiles, pfx, c):
                """bn stats for a 4-tile chunk; rstd = rsqrt(var+eps)
                entirely on the DVE (magic-seed + 2 Newton steps, ~5e-6 rel
                err) so ACT never loads the Ln table - it stays
                exp-table-resident for the whole kernel. Per-chunk so loop2
                can start before all of loop1 finishes."""
                nt = len(src_tiles)
                mv_all = work.tile([128, 2 * nt], F32, tag=f"{pfx}mv",
                                   bufs=NCH, name=f"{pfx}mv{c}")
                for t, x_t in enumerate(src_tiles):
                    stats = work.tile([128, 6], F32, tag="stats", bufs=4)
                    nc.vector.bn_stats(out=stats, in_=x_t)
                    nc.vector.bn_aggr(out=mv_all[:, 2 * t:2 * t + 2],
                                      in_=stats)
                rstd = work.tile([128, nt], F32, tag=f"{pfx}rstd", bufs=NCH,
                                 name=f"{pfx}rstd{c}")
                ve = work.tile([128, nt], F32, tag=f"{pfx}ve", bufs=2)
                a = work.tile([128, nt], F32, tag=f"{pfx}nra", bufs=2)
                b = work.tile([128, nt], F32, tag=f"{pfx}nrb", bufs=2)
                nc.vector.tensor_scalar_add(ve, mv_all[:, 1::2], EPS)
                # seed bits: magic - (u>>1) == ~(u>>1) + (magic+1); int32 so
                # no unsigned saturation (DVE int ALU saturates on overflow)
                nc.vector.tensor_scalar(
                    rstd.bitcast(I32), ve.bitcast(I32),
                    scalar1=1, scalar2=0xFFFFFFFF,
                    op0=ALU.logical_shift_right, op1=ALU.bitwise_xor)
                nc.vector.tensor_scalar_add(rstd.bitcast(I32),
                                            rstd.bitcast(I32), 0x5f3759e0)
                for _ in range(2):
                    nc.vector.tensor_tensor(a, rstd, rstd, op=ALU.mult)
                    nc.vector.tensor_tensor(b, a, ve, op=ALU.mult)
                    nc.vector.tensor_scalar(a, b, scalar1=-0.5, scalar2=1.5,
                                            op0=ALU.mult, op1=ALU.add)
                    nc.vector.tensor_tensor(rstd, a, rstd, op=ALU.mult)
                # mb = -mean*rstd so ln_apply can run on ACT as
                # Identity(x*rstd + mb)
                mb = work.tile([128, nt], F32, tag=f"{pfx}mb", bufs=NCH,
                               name=f"{pfx}mb{c}")
                nc.vector.scalar_tensor_tensor(
                    mb, mv_all[:, 0::2], -1.0, rstd,
                    op0=ALU.mult, op1=ALU.mult)
                return mb, rstd

            def ln_apply(x_t, mb_c, rstd_c):
                h_t = work.tile([128, C], F32R, tag="h", bufs=5)
                nc.scalar.activation(h_t, x_t, ACT.Identity,
                                     bias=mb_c, scale=rstd_c)
                return h_t

            # ---- LN1 stats, per chunk --------------------------------------
            x_tiles = []
            for cc in range(NCH):
                xall = work.tile([128, 4 * C], F32, tag="x", bufs=NCH,
                                 name=f"xall{cc}")
                src = xf[cc * 512:(cc + 1) * 512]
                nc.sync.dma_start(
                    out=xall.rearrange("p (j c) -> p j c", j=4),
                    in_=src.rearrange("(j p) c -> p j c", p=128))
                x_tiles += [xall[:, j * C:(j + 1) * C] for j in range(4)]
                stats_c = ln_stats_chunk(x_tiles[4 * cc:4 * cc + 4], "a", cc)
                if cc == 0:
                    stats1 = [stats_c]
                else:
                    stats1.append(stats_c)

            # ---- weights ---------------------------------------------------
            def load_w(name, dram_ap):
                """Load a [C, C]-layout weight as ONE [128, NK*C] tile in a
                single DMA on the ACT HWDGE queue (parallel to x on SP;
                fewer descriptors - issue rate, not bandwidth, limits the
                startup). k-tile k = columns [k*C, (k+1)*C). f32r is
                bit-identical to f32, so bitcast the DRAM side."""
                wt = consts.tile([128, NK * C], F32R, tag=f"{name}",
                                 name=f"{name}")
                src = dram_ap.rearrange("(k p) c -> p k c", p=128)
                dst = wt.rearrange("p (k c) -> p k c", k=NK)
                nc.sync.dma_start(out=dst, in_=src.bitcast(F32R))
                return [wt[:, k * C:(k + 1) * C] for k in range(NK)]

            wq_t = load_w("wq", wq.ap())
            wk_t = load_w("wk", wk.ap())
            wv_t = load_w("wv", wv.ap())
            wp_t = load_w("wp", wp.ap())
            w1_t = load_w("w1", w1.ap())
            w2_t = load_w("w2", w2.ap())

            def load_cols(name, dram_handle):
                """[C]-flat bias -> [128, NM] tile, one DMA; col m is the
                m-th 128-row block of the bias."""
                flat = dram_handle.ap().rearrange("h d -> (h d)") \
                    if len(dram_handle.shape) == 2 else dram_handle.ap()
                t = consts.tile([128, NM], F32, tag=f"{name}",
                                name=f"{name}")
                nc.sync.dma_start(out=t,
                                  in_=flat.rearrange("(m p) -> p m", p=128))
                return [t[:, m:m + 1] for m in range(NM)]

            bq_c = load_cols("bq", bq)
            bk_c = load_cols("bk", bk)
            b1_c = load_cols("b1", b1)

            # bp/b2/bv as single-partition f32r rows; applied via a K=1
            # ones-row matmul folded into the PSUM accumulation (keeps bias
            # off the DVE).
            ones_r = consts.tile([1, 128], F32R, tag="ones_r")
            ones32 = consts.tile([1, 128], F32, tag="ones32")
            nc.vector.memset(ones32, 1.0)
            nc.vector.tensor_copy(ones_r, ones32)

            # all-ones [128, 64] stationary: Z[t] = sum_s expT[s, t]
            # replicated into 64 PSUM partitions in one matmul
            ones64 = consts.tile([128, 64], F32R, tag="ones64")
            ones64_32 = work.tile([128, 64], F32, tag="ones64_32", bufs=1)
            nc.vector.memset(ones64_32, 1.0)
            nc.vector.tensor_copy(ones64, ones64_32)

            def load_row1(name, handle):
                t = consts.tile([1, C], F32R, tag=f"{name}r1", name=f"{name}r1")
                flat = handle.ap().rearrange("h d -> (h d)") \
                    if len(handle.shape) == 2 else handle.ap()
                nc.sync.dma_start(out=t, in_=flat.bitcast(F32R))
                return t

            bp_r1 = load_row1("bp", bp)
            b2_r1 = load_row1("b2", b2)
            bv_r1 = load_row1("bv", bv)

            # ---- helpers ---------------------------------------------------
            # ---- Loop 1, software-pipelined over chunks ----------------
            # Stage A (LN1 normalize, hT transposes, qT/kT/v projections)
            # for chunk c+1 is emitted interleaved between the attention
            # units of chunk c, so the FIFO PE stream has independent
            # matmuls to run during each unit's DVE/ACT/Pool dependency
            # stalls.
            def stageA(c):
                st = {"hT": [None] * NK, "q": [None] * NM,
                      "k": [None] * NM, "v": [None] * 4}
                parts = []

                def p_h():
                    mb1, rstd1 = stats1[c]
                    st["h"] = [
                        ln_apply(x_tiles[4 * c + j], mb1[:, j:j + 1],
                                 rstd1[:, j:j + 1])
                        for j in range(4)]
                parts.append(p_h)

                def mk_tr(k):
                    def p():
                        pst = ps.tile([128, 512], F32R, tag="pacc", bufs=5,
                                      name=f"pstr{k}")
                        for j in range(4):
                            nc.tensor.transpose(
                                pst[:, j * 128:(j + 1) * 128],
                                st["h"][j][:, k * 128:(k + 1) * 128], ident)
                        sb = work.tile([128, 512], F32R, tag="hT", bufs=6,
                                       name=f"hT_{k}_{c}")
                        nc.scalar.activation(sb, pst, ACT.Copy, bias=0.0)
                        st["hT"][k] = sb
                    return p
                parts += [mk_tr(k) for k in range(NK)]

                def mk_qk(w_tiles, bias_cols, key, tag, m):
                    def p():
                        acc = ps.tile([128, 512], F32, tag="pacc", bufs=5)
                        for k in range(NK):
                            nc.tensor.matmul(
                                acc, w_tiles[k][:, m * 128:(m + 1) * 128],
                                st["hT"][k], start=(k == 0),
                                stop=(k == NK - 1))
                        sb = work.tile([128, 512], F32R, tag=tag, bufs=6,
                                       name=f"{tag}_{m}_{c}")
                        # evacuate on ACT with the bias folded in (Identity
                        # is in the Exp table set - no table reload)
                        nc.scalar.activation(sb, acc, ACT.Identity,
                                             bias=bias_cols[m])
                        st[key][m] = sb
                    return p
                parts += [mk_qk(wq_t, bq_c, "q", "qT", m) for m in range(NM)]
                parts += [mk_qk(wk_t, bk_c, "k", "kT", m) for m in range(NM)]

                def mk_v(j):
                    def p():
                        acc = ps.tile([128, C], F32, tag="pacc", bufs=5)
                        for k in range(NK):
                            nc.tensor.matmul(
                                acc, st["hT"][k][:, j * 128:(j + 1) * 128],
                                wv_t[k], start=(k == 0), stop=False)
                        nc.tensor.matmul(acc, ones_r, bv_r1,
                                         start=False, stop=True)
                        v_t = work.tile([128, C], F32R, tag="v", bufs=8)
                        nc.scalar.activation(v_t, acc, ACT.Copy, bias=0.0)
                        st["v"][j] = v_t
                    return p
                parts += [mk_v(j) for j in range(4)]
                return st, parts

            x2_tiles = [None] * NT
            stats2 = [None] * NCH

            def loop2_parts(c):
                """FFN for chunk c (h2T transposes, ff1, ff2+store) as parts
                interleaved into chunk c+1's attention units."""
                st2 = {"h2T": [None] * NK, "ff1": [None] * NM}
                parts = []

                def p_h2():
                    mb2, rstd2 = stats2[c]
                    st2["h2"] = [
                        ln_apply(x2_tiles[4 * c + j],
                                 mb2[:, j:j + 1], rstd2[:, j:j + 1])
                        for j in range(4)]
                parts.append(p_h2)

                def mk_tr2(k):
                    def p():
                        pst = ps.tile([128, 512], F32R, tag="pacc", bufs=5,
                                      name=f"pstr{k}")
                        for j in range(4):
                            nc.tensor.transpose(
                                pst[:, j * 128:(j + 1) * 128],
                                st2["h2"][j][:, k * 128:(k + 1) * 128],
                                ident)
                        sb = work.tile([128, 512], F32R, tag="hT", bufs=6,
                                       name=f"h2T_{k}_{c}")
                        nc.scalar.activation(sb, pst, ACT.Copy, bias=0.0)
                        st2["h2T"][k] = sb
                    return p
                parts += [mk_tr2(k) for k in range(NK)]

                def mk_ff1(m):
                    def p():
                        acc = ps.tile([128, 512], F32, tag="pacc", bufs=5)
                        for k in range(NK):
                            nc.tensor.matmul(
                                acc, w1_t[k][:, m * 128:(m + 1) * 128],
                                st2["h2T"][k], start=(k == 0),
                                stop=(k == NK - 1))
                        sb = work.tile([128, 512], F32R, tag="qT", bufs=6,
                                       name=f"ff1T_{m}_{c}")
                        nc.scalar.activation(sb, acc, ACT.Relu, bias=b1_c[m])
                        st2["ff1"][m] = sb
                    return p
                parts += [mk_ff1(m) for m in range(NM)]

                def mk_ff2(j):
                    def p():
                        t = 4 * c + j
                        acc = ps.tile([128, C], F32, tag="pacc", bufs=5)
                        for k in range(NK):
                            nc.tensor.matmul(
                                acc,
                                st2["ff1"][k][:, j * 128:(j + 1) * 128],
                                w2_t[k], start=(k == 0), stop=False)
                        nc.tensor.matmul(acc, ones_r, b2_r1,
                                         start=False, stop=True)
                        y_t = work.tile([128, C], F32, tag="y", bufs=3)
                        nc.vector.scalar_tensor_tensor(
                            y_t, acc, 1.0, x2_tiles[t],
                            op0=ALU.mult, op1=ALU.add)
                        nc.sync.dma_start(out=yf[t * 128:(t + 1) * 128],
                                          in_=y_t)
                    return p
                parts += [mk_ff2(j) for j in range(4)]
                return parts

            stc, parts0 = stageA(0)
            for p in parts0:
                p()
            pending = []
            for c in range(NCH):
                if c + 1 < NCH:
                    next_st, pending = stageA(c + 1)
                else:
                    next_st, pending = None, []
                if c >= 1:
                    pending = pending + loop2_parts(c - 1)
                n_parts = len(pending)
                emitted = 0
                uidx = 0
                for b in (2 * c, 2 * c + 1):
                    off_b = (b % 2) * 256
                    attnTb = [None] * NM
                    for hp in range(H // 2):
                        # head pair (2hp, 2hp+1) at row offsets 0/64: the
                        # scores matmuls use K=64 stationaries in different
                        # row groups, so adjacent emission lets the PE run
                        # them concurrently (tile-position packing).
                        sps = [None, None]
                        for hi in range(2):
                            off = 64 * hi
                            sp = ps.tile([128, 384], F32, tag="punit",
                                         bufs=3, name=f"sps{hi}")
                            nc.tensor.matmul(
                                sp[:, 0:256],
                                stc["k"][hp][off:off + 64,
                                             off_b:off_b + 128],
                                stc["q"][hp][off:off + 64,
                                             off_b:off_b + 256],
                                start=True, stop=False)
                            nc.tensor.matmul(
                                sp[:, 256:384],
                                stc["k"][hp][off:off + 64,
                                             off_b + 128:off_b + 256],
                                stc["q"][hp][off:off + 64,
                                             off_b + 128:off_b + 256],
                                start=False, stop=False)
                            sps[hi] = sp
                        expTs = [None, None]
                        for hi in range(2):
                            nc.tensor.matmul(sps[hi], ident, maskF,
                                             start=False, stop=True)
                            expT = work.tile([128, 384], F32R, tag="expT",
                                             bufs=6)
                            nc.scalar.activation(expT, sps[hi], ACT.Exp,
                                                 bias=0.0, scale=SCALE)
                            expTs[hi] = expT
                        # shared PSUM banks for the pair: z_ps rows = Z[t]
                        # replicated (ones64 stationary), attn_ps cols
                        # 0:256 / 256:512 = heads 2hp / 2hp+1. Z first so
                        # the DVE recip overlaps the v matmuls.
                        z_ps = ps.tile([64, 512], F32, tag="pacc", bufs=5,
                                       name="psz")
                        attn_ps = ps.tile([64, 512], F32, tag="pacc",
                                          bufs=5, name="psa")
                        for hi in range(2):
                            nc.tensor.matmul(z_ps[:, hi * 256:hi * 256 + 256],
                                             ones64, expTs[hi][:, 0:256],
                                             start=(hi == 0), stop=False)
                            nc.tensor.matmul(
                                z_ps[:, hi * 256 + 128:hi * 256 + 256],
                                ones64, expTs[hi][:, 256:384],
                                start=False, stop=(hi == 1))
                        for hi in range(2):
                            off = 64 * hi
                            nc.tensor.matmul(
                                attn_ps[:, hi * 256:hi * 256 + 256],
                                stc["v"][2 * (b % 2)][
                                    :, hp * 128 + off:hp * 128 + off + 64],
                                expTs[hi][:, 0:256],
                                start=(hi == 0), stop=False)
                            nc.tensor.matmul(
                                attn_ps[:, hi * 256 + 128:hi * 256 + 256],
                                stc["v"][2 * (b % 2) + 1][
                                    :, hp * 128 + off:hp * 128 + off + 64],
                                expTs[hi][:, 256:384],
                                start=False, stop=(hi == 1))
                        rz = work.tile([64, 512], F32, tag="rz", bufs=6)
                        nc.vector.reciprocal_approx_fast(rz, z_ps)
                        attnTb[hp] = work.tile(
                            [128, 256], F32R, tag="attnT", bufs=9,
                            name=f"attnT_{hp}_{b}")
                        # normalize fused into the PSUM->SBUF evacuation
                        for hi in range(2):
                            nc.vector.scalar_tensor_tensor(
                                attnTb[hp][64 * hi:64 * hi + 64, :],
                                attn_ps[:, hi * 256:hi * 256 + 256],
                                1.0, rz[:, hi * 256:hi * 256 + 256],
                                op0=ALU.mult, op1=ALU.mult)
                        # interleave next chunk's stage-A / prev FFN parts
                        uidx += 1
                        want = (n_parts * uidx + 5) // 6
                        while pending and emitted < want:
                            pending.pop(0)()
                            emitted += 1
                    # projection + residual for t = 2b, 2b+1
                    for j in range(2):
                        t = 2 * b + j
                        acc = ps.tile([128, C], F32, tag="pacc", bufs=5)
                        for k in range(NK):
                            nc.tensor.matmul(
                                acc, attnTb[k][:, j * 128:(j + 1) * 128],
                                wp_t[k], start=(k == 0), stop=False)
                        nc.tensor.matmul(acc, ones_r, bp_r1,
                                         start=False, stop=True)
                        x2_t = work.tile([128, C], F32, tag="x2", bufs=NT,
                                         name=f"x2_{t}")
                        nc.vector.scalar_tensor_tensor(
                            x2_t, acc, 1.0, x_tiles[t],
                            op0=ALU.mult, op1=ALU.add)
                        x2_tiles[t] = x2_t
                # ---- LN2 stats for this chunk (x2 tiles just completed)
                stats2[c] = ln_stats_chunk(x2_tiles[4 * c:4 * c + 4], "b", c)
                for p in pending:
                    p()
                stc = next_st
            # FFN for the final chunk (nothing left to interleave with)
            for p in loop2_parts(NCH - 1):
                p()


        if loop_n is None:
            emit_body()
        else:
            with tc.For_i(0, loop_n, 1):
                emit_body()
    nc.compile()
    return nc


_NC_CACHE = None


def _get_nc():
    global _NC_CACHE
    if _NC_CACHE is None:
        _NC_CACHE = build()
    return _NC_CACHE


def _fold_ln(inputs):
    """Fold LN gamma/beta into the downstream weights (host-side, fp32)."""
    f = {k: np.asarray(v, dtype=np.float32) for k, v in inputs.items()}
    g1, be1 = f["ln1_g"], f["ln1_b"]
    g2, be2 = f["ln2_g"], f["ln2_b"]
    out = dict(f)
    for wn, bn in (("Wq", "bq"), ("Wk", "bk"), ("Wv", "bv")):
        w = f[wn]  # [H, C, D]
        out[wn] = w * g1[None, :, None]
        out[bn] = f[bn] + np.einsum("c,hcd->hd", be1, w)
    out["W1"] = f["W1"] * g2[:, None]
    out["b1"] = f["b1"] + be2 @ f["W1"]
    return out


def device_base_inputs(inputs):
    """Fold LN into weights and lay Wq/Wk/Wv out as [C, H*D] c-major."""
    f = _fold_ln(inputs)
    for wn in ("Wq", "Wk", "Wv"):
        f[wn] = f[wn].transpose(1, 0, 2).reshape(C, C)
    names = ["Wq", "Wk", "Wv", "bq", "bk", "bv", "Wp", "bp",
             "W1", "b1", "W2", "b2"]
    return {n: np.ascontiguousarray(f[n]) for n in names}


def kernel(**inputs):
    nc = _get_nc()
    x = np.asarray(inputs["x"], dtype=np.float32)
    base = device_base_inputs(inputs)
    in_maps = []
    for i in range(N_CORES):
        m = dict(base)
        m["x"] = np.ascontiguousarray(x[i * B_CORE:(i + 1) * B_CORE])
        in_maps.append(m)
    r = run_bass_kernel_spmd(nc, in_maps, core_ids=list(range(N_CORES)))
    out = np.concatenate([r.results[i]["out"] for i in range(N_CORES)], axis=0)
    return out.astype(np.float32)


if __name__ == "__main__":
    nc = build()
    print("build ok")

